# revision 4
# baseline (speedup 1.0000x reference)
"""EulerAttention Trainium2 kernel.

Full inputs -> full outputs; internally shards batch B=16 across 8 NeuronCores
(2 batches per core). Single Bass program run SPMD via run_bass_kernel_spmd.

Device pipeline per [128 s x 2048 (2 s-chunks x 1024 d)] tile:
    t_a = cs * (c1*A)_row                        (DVE tensor_tensor mult)
    ty  = t_a - (rq + 1024 - MAGIC)_row          (GPSIMD TT sub; rounds to int+MAGIC)
    vi  = bits(ty) AND 4095                      (DVE tensor_scalar int)
    cd  = Sin(c2*vi - pi)  == cos(theta_q^-theta_k^)   (ACT)
    scores = segmented sum over d (16 heads)     (DVE tensor_reduce, 4D AP)
Then per batch: exp (ACT; no max needed, scores bounded by +-5.66), PE-accumulated
context numerator/denominator, normalize, head-sum via PE, exact LUT output stage.
"""
import sys, math

sys.path.insert(0, "/opt/trn_rl_repo")
import numpy as np

B, S, D, H = 16, 4096, 1024, 16
DH = D // H
N_CORES = 8
B_LOC = B // N_CORES             # 2
KC = 2                           # s-chunks per tile
N_T = S // (128 * KC)            # 16 tiles per batch
FW = KC * D                      # 2048 free width
LUT = 4096
PHI = (1.0 + math.sqrt(5.0)) / 2.0
TWO_PI = 2.0 * math.pi
C1 = np.float32(LUT / TWO_PI)
C2 = np.float32(TWO_PI / LUT)
MAGIC = np.float32(1.5 * 2 ** 23)
PI_F = float(np.float32(math.pi))
INV_TAU = float(np.float32(1.0 / math.sqrt(2.0 * DH)))
SUB_ENGINE = "gpsimd"            # "vector" | "gpsimd"

_CACHE = {}


def _reg_const(nc, mybir, val):
    val = float(val)
    t = nc.alloc_sbuf_tensor(f"const-f32-{val}", [128, 1], mybir.dt.float32)
    nc.gpsimd.memset(t.ap(), val)
    nc.const_aps.aps[(mybir.dt.float32, val)] = t.ap()


def build_program():
    from concourse import bacc, tile, mybir

    nc = bacc.Bacc("TRN2", target_bir_lowering=False, debug=False)
    _reg_const(nc, mybir, -PI_F)
    nc.all_engine_barrier()

    f32 = mybir.dt.float32
    i32 = mybir.dt.int32
    AT = mybir.ActivationFunctionType
    OP = mybir.AluOpType

    cs_d = nc.dram_tensor("cs", [B_LOC, S, D], f32, kind="ExternalInput").ap()
    c1a_d = nc.dram_tensor("c1a", [128, FW], f32, kind="ExternalInput").ap()
    rq_d = nc.dram_tensor("rqrow", [B_LOC, 128, FW], f32, kind="ExternalInput").ap()
    ones_d = nc.dram_tensor("ones", [128, 1], f32, kind="ExternalInput").ap()
    rot_d = nc.dram_tensor("roT", [128, 8], f32, kind="ExternalInput").ap()
    bot_d = nc.dram_tensor("boT", [B_LOC, 128, 8], f32, kind="ExternalInput").ap()
    ost_d = nc.dram_tensor("osT", [128, 8], f32, kind="ExternalInput").ap()
    out_d = nc.dram_tensor("out", [B_LOC, D], f32, kind="ExternalOutput").ap()

    with tile.TileContext(nc) as tc:
        from contextlib import ExitStack
        with ExitStack() as ctx:
            cpool = ctx.enter_context(tc.tile_pool(name="cs", bufs=N_T))
            wpool = ctx.enter_context(tc.tile_pool(name="work", bufs=2))
            spool = ctx.enter_context(tc.tile_pool(name="small", bufs=2))
            kpool = ctx.enter_context(tc.tile_pool(name="konst", bufs=1))
            ppool = ctx.enter_context(tc.tile_pool(name="psum", bufs=1, space="PSUM"))

            c1a = kpool.tile([128, FW], f32, name="c1a_t")
            nc.sync.dma_start(c1a[:, :], c1a_d)
            rqr = [kpool.tile([128, FW], f32, name=f"rqr{b}") for b in range(B_LOC)]
            for b in range(B_LOC):
                nc.sync.dma_start(rqr[b][:, :], rq_d[b])
            ones = kpool.tile([128, 1], f32, name="ones_t")
            nc.sync.dma_start(ones[:, :], ones_d)
            rot = kpool.tile([128, 8], f32, name="rot_t")
            nc.sync.dma_start(rot[:, :], rot_d)
            bot = [kpool.tile([128, 8], f32, name=f"bot{b}") for b in range(B_LOC)]
            for b in range(B_LOC):
                nc.sync.dma_start(bot[b][:, :], bot_d[b])
            ost = kpool.tile([128, 8], f32, name="ost_t")
            nc.sync.dma_start(ost[:, :], ost_d)

            sub_eng = nc.gpsimd if SUB_ENGINE == "gpsimd" else nc.vector

            for b in range(B_LOC):
                cs_tiles = []
                scores = spool.tile([128, S // 128 * H], f32, name="scores", tag="scores")
                for t in range(N_T):
                    cst = cpool.tile([128, FW], f32, name=f"cs_{b}_{t}", tag="cs")
                    # [128 p, KC k, 1024 d] <- cs[b, t*KC*128 + k*128 + p, d]
                    src = cs_d[b, t * KC * 128:(t + 1) * KC * 128, :].rearrange(
                        "(k p) d -> p k d", p=128)
                    nc.sync.dma_start(cst[:, :].rearrange("p (k d) -> p k d", k=KC), src)
                    cs_tiles.append(cst)
                    ta = wpool.tile([128, FW], f32, name=f"ta_{b}_{t}", tag="w1")
                    nc.vector.tensor_tensor(ta[:, :], cst[:, :], c1a[:, :], OP.mult)
                    ty = wpool.tile([128, FW], f32, name=f"ty_{b}_{t}", tag="w2")
                    sub_eng.tensor_tensor(ty[:, :], ta[:, :], rqr[b][:, :], OP.subtract)
                    vi = wpool.tile([128, FW], i32, name=f"vi_{b}_{t}", tag="w1")
                    nc.vector.tensor_scalar(vi[:, :], ty[:, :].bitcast(i32),
                                            4095, None, OP.bitwise_and)
                    cd = wpool.tile([128, FW], f32, name=f"cd_{b}_{t}", tag="w2")
                    nc.scalar.activation(cd[:, :], vi[:, :], AT.Sin,
                                         bias=-PI_F, scale=float(C2))
                    nc.vector.tensor_reduce(
                        scores[:, t * KC * H:(t + 1) * KC * H],
                        cd[:, :].rearrange("p (k h d) -> p k h d", k=KC, h=H),
                        mybir.AxisListType.X, OP.add)

                p = spool.tile([128, S // 128 * H], f32, name="p", tag="p")
                nc.scalar.activation(p[:, :], scores[:, :], AT.Exp, scale=INV_TAU)

                num0 = ppool.tile([16, 512], f32, name=f"num0_{b}", tag="num0")
                num1 = ppool.tile([16, 512], f32, name=f"num1_{b}", tag="num1")
                den = ppool.tile([16, 1], f32, name=f"den_{b}", tag="den")
                n_grp = N_T * KC
                for t in range(N_T):
                    for k in range(KC):
                        g = t * KC + k
                        lhs = p[:, g * H:(g + 1) * H]
                        kw = dict(start=(g == 0), stop=(g == n_grp - 1))
                        nc.tensor.matmul(num0[:, :], lhs,
                                         cs_tiles[t][:, k * D:k * D + 512], **kw)
                        nc.tensor.matmul(num1[:, :], lhs,
                                         cs_tiles[t][:, k * D + 512:(k + 1) * D], **kw)
                        nc.tensor.matmul(den[:, :], lhs, ones[:, :], **kw)

                rec = spool.tile([16, 1], f32, name=f"rec_{b}", tag="rec")
                nc.vector.reciprocal(rec[:, :], den[:, :])
                ctxn = spool.tile([16, D], f32, name=f"ctxn_{b}", tag="ctxn")
                nc.vector.tensor_scalar(ctxn[:, 0:512], num0[:, :], rec[:, :], None, OP.mult)
                nc.vector.tensor_scalar(ctxn[:, 512:1024], num1[:, :], rec[:, :], None, OP.mult)

                ctxT = ppool.tile([128, 8], f32, name=f"ctxT_{b}", tag="ctxT")
                for c in range(8):
                    nc.tensor.matmul(ctxT[:, c:c + 1],
                                     ctxn[:, c * 128:(c + 1) * 128],
                                     ones[0:16, :], start=True, stop=True)

                m1 = spool.tile([128, 8], f32, name=f"m1_{b}", tag="m1")
                nc.vector.tensor_tensor(m1[:, :], ctxT[:, :], rot[:, :], OP.mult)
                m2 = spool.tile([128, 8], f32, name=f"m2_{b}", tag="m2")
                nc.vector.tensor_tensor(m2[:, :], m1[:, :], bot[b][:, :], OP.add)
                yo = spool.tile([128, 8], f32, name=f"yo_{b}", tag="yo")
                nc.vector.tensor_scalar(yo[:, :], m2[:, :],
                                        float(MAGIC) + 2560.0, None, OP.add)
                vo = spool.tile([128, 8], i32, name=f"vo_{b}", tag="vo")
                nc.vector.tensor_scalar(vo[:, :], yo[:, :].bitcast(i32),
                                        4095, None, OP.bitwise_and)
                sp = spool.tile([128, 8], f32, name=f"sp_{b}", tag="sp")
                nc.scalar.activation(sp[:, :], vo[:, :], AT.Sin,
                                     bias=-PI_F, scale=float(C2))
                ot = spool.tile([128, 8], f32, name=f"ot_{b}", tag="ot")
                nc.vector.tensor_tensor(ot[:, :], sp[:, :], ost[:, :], OP.mult)
                nc.sync.dma_start(out_d[b].rearrange("(c p) -> p c", p=128), ot[:, :])

    nc.compile()
    return nc


def _host_prep(x, t, w_query, b_query, w_key, b_key, w_out, b_out, out_scale):
    f = np.float32
    xh = x.reshape(B, H, DH).astype(f)
    t_phi = (t.astype(f) * f(PHI)).astype(f)
    theta_q = ((xh / (f(1.0) + np.abs(w_query.astype(f)))).astype(f)
               + b_query.astype(f)).astype(f)
    theta_q = (theta_q + t_phi[:, None, None]).astype(f)
    rq = np.round((theta_q * C1).astype(f)).astype(np.float64).reshape(B, D)

    a_key = 1.0 / (1.0 + np.abs(w_key.astype(np.float64)))
    c1a_row = (float(C1) * a_key).reshape(D).astype(f)
    c1a = np.broadcast_to(np.tile(c1a_row, KC), (128, FW)).copy()
    c1bk = (float(C1) * b_key.astype(np.float64)).reshape(D)
    rq_row = (rq + 1024.0 - np.float64(MAGIC) - c1bk[None, :])     # [B, D]
    rq_rep = np.broadcast_to(np.tile(rq_row, (1, KC))[:, None, :].astype(f),
                             (B, 128, FW)).copy()

    ones = np.ones((128, 1), f)
    m_idx = (np.arange(8)[None, :] * 128 + np.arange(128)[:, None])
    a_out = (1.0 / (1.0 + np.abs(w_out.astype(np.float64)))).reshape(D)
    roT = (float(C1) * a_out[m_idx]).astype(f)
    t_phi64 = t_phi.astype(np.float64)
    boT = np.empty((B, 128, 8), f)
    for b in range(B):
        boT[b] = (float(C1) * (b_out.astype(np.float64).reshape(D)[m_idx]
                               + t_phi64[b])).astype(f)
    osT = (out_scale.astype(np.float64).reshape(D)[m_idx]
           * math.sqrt(2.0)).astype(f)
    return c1a, rq_rep, ones, roT, boT, osT


def _make_in_maps(inputs):
    c1a, rq_rep, ones, roT, boT, osT = _host_prep(
        inputs["x"], inputs["t"], inputs["w_query"], inputs["b_query"],
        inputs["w_key"], inputs["b_key"], inputs["w_out"], inputs["b_out"],
        inputs["out_scale"])
    in_maps = []
    for c in range(N_CORES):
        b0 = c * B_LOC
        in_maps.append({
            "cs": np.ascontiguousarray(inputs["cached_states"][b0:b0 + B_LOC]).astype(np.float32),
            "c1a": c1a,
            "rqrow": np.ascontiguousarray(rq_rep[b0:b0 + B_LOC]),
            "ones": ones,
            "roT": roT,
            "boT": np.ascontiguousarray(boT[b0:b0 + B_LOC]),
            "osT": osT,
        })
    return in_maps


def kernel(x, cached_states, t, w_query, b_query, w_key, b_key, w_out, b_out,
           out_scale):
    from concourse.bass_utils import run_bass_kernel_spmd

    inputs = dict(x=np.asarray(x), cached_states=np.asarray(cached_states),
                  t=np.asarray(t), w_query=np.asarray(w_query),
                  b_query=np.asarray(b_query), w_key=np.asarray(w_key),
                  b_key=np.asarray(b_key), w_out=np.asarray(w_out),
                  b_out=np.asarray(b_out), out_scale=np.asarray(out_scale))
    if "nc" not in _CACHE:
        _CACHE["nc"] = build_program()
    nc = _CACHE["nc"]
    in_maps = _make_in_maps(inputs)
    res = run_bass_kernel_spmd(nc, in_maps, core_ids=list(range(N_CORES)))
    out = np.concatenate([res.results[c]["out"] for c in range(N_CORES)], axis=0)
    return out.astype(np.float32)
